# revision 9
# baseline (speedup 1.0000x reference)
"""Walsh-Hadamard transform (last dim 4096) on 8 Trainium2 NeuronCores.

Input x: (4, 2048, 4096) fp32. Output: fwht(x) * 1/sqrt(4096), where fwht is
the Sylvester-Hadamard transform H_4096 @ row.

Math: H_4096 = H_16 (x) H_256 (Kronecker). Per row reshaped to X (16 x 256):
    Y = (H16/8) @ X @ (H256/8)          (1/64 = 1/sqrt(4096) split exactly)
Row-major layout: row[e], e = i1*256 + i2  ->  X[i1, i2]; output identical.

On TensorE (out = lhsT.T @ rhs, lhsT stationary):
  pass 1: lhsT = 8-row data tile [(kb,i1) x (i2 half)], rhs = blockdiag_8(H16/8)
          -> out = Z^T  (partition = i2, free = (nb rows, j1))
  pass 2: lhsT = Z^T halves, rhs = H256/8 K-slabs, accumulate -> Y natural
The data passes through the PE as the *stationary* operand both times; the
implicit transpose of lhsT cancels, so no transpose instructions are needed,
and every DMA chunk is >= 512B contiguous.

Sharding: 8192 rows data-parallel -> 1024 contiguous rows per core.
"""

import os
import sys

sys.path.insert(0, "/opt/trn_rl_repo")

import numpy as np

import concourse.bacc as bacc
import concourse.mybir as mybir
import concourse.tile as tile
from concourse.bass_utils import run_bass_kernel_spmd

N_CORES = 8
ROWS_PER_CORE = 1024
N_LAST = 4096
I1, I2 = 16, 256          # H_4096 = H_16 (x) H_256
KB = 8                    # rows per matmul group (8*16 = 128 partitions)
GROUPS = ROWS_PER_CORE // KB          # 128 groups/core
G_SUPER = 8                           # groups per DMA super-block (64 rows)
SUPERS = GROUPS // G_SUPER            # 8

USE_FP32R = os.environ.get("HAD_FP32", "") != "1"   # fp32r: 1 cyc/row PE path


def _hadamard(n):
    h = np.array([[1.0]], dtype=np.float64)
    while h.shape[0] < n:
        h = np.block([[h, h], [h, -h]])
    return h


def _build_consts():
    h16 = _hadamard(I1) / 8.0
    h256 = _hadamard(I2) / 8.0
    bd = np.kron(np.eye(KB), h16)                      # [128, 128]
    if USE_FP32R:
        hbd = np.concatenate([bd, bd], axis=1)         # [128, 256] dup for N>=256
    else:
        hbd = bd
    return hbd.astype(np.float32), h256.astype(np.float32)


_CACHED_NC = None


def _build_program():
    global _CACHED_NC
    if _CACHED_NC is not None:
        return _CACHED_NC

    f32 = mybir.dt.float32
    f32r = mybir.dt.float32r
    mm_dt = f32r if USE_FP32R else f32
    n_dup = 256 if USE_FP32R else 128

    nc = bacc.Bacc(None, target_bir_lowering=False, debug=False)
    x = nc.declare_dram_parameter("x", [ROWS_PER_CORE, N_LAST], mm_dt, isOutput=False)
    hbd = nc.declare_dram_parameter("hbd", [128, n_dup], mm_dt, isOutput=False)
    h256 = nc.declare_dram_parameter("h256", [I2, I2], mm_dt, isOutput=False)
    y = nc.declare_dram_parameter("y", [ROWS_PER_CORE, N_LAST], f32, isOutput=True)

    # DRAM views. Partition stride is uniform: addr = p*256 + h*128 + i2 within
    # a group (p = kb*16 + i1), so the partition dim collapses to one stride.
    xr = x.rearrange(
        "(s g kb) (i1 i2) -> s (kb i1) g i2",
        s=SUPERS, g=G_SUPER, kb=KB, i1=I1, i2=I2,
    )   # [S, 128, G, 256] — per (partition, g): 1KB contiguous; per g: 128KB run
    yr = y.rearrange(
        "(s g nb) (j1 j2) -> s (nb j1) g j2",
        s=SUPERS, g=G_SUPER, nb=KB, j1=I1, j2=I2,
    )   # [8, 128, 16, 256]

    with tile.TileContext(nc) as tc:
        with (
            tc.tile_pool(name="consts", bufs=1) as cpool,
            tc.tile_pool(name="xin", bufs=10) as xpool,
            tc.tile_pool(name="zt", bufs=8) as zpool,
            tc.tile_pool(name="yout", bufs=4) as ypool,
            tc.tile_pool(name="ps1", bufs=4, space="PSUM") as ps1pool,
            tc.tile_pool(name="ps2", bufs=4, space="PSUM") as ps2pool,
        ):
            hbd_t = cpool.tile([128, n_dup], mm_dt)
            nc.scalar.dma_start(hbd_t[:], hbd[:])
            h256_t = cpool.tile([128, 2, I2], mm_dt)
            nc.scalar.dma_start(
                h256_t[:],
                h256.rearrange("(h k) j -> k h j", h=2, k=128),
            )

            hbd_r = hbd_t[:]

            for s in range(SUPERS):
                xt = xpool.tile([128, G_SUPER * I2], mm_dt, tag="xin", name=f"xt{s}")
                nc.sync.dma_start(
                    xt[:].rearrange("p (g i) -> p g i", g=G_SUPER),
                    xr[s],
                )
                yt = ypool.tile([128, G_SUPER * I2], f32, tag="yout")
                for g in range(G_SUPER):
                    ps1 = ps1pool.tile([128, 2, n_dup], f32, tag="ps1")
                    for h in range(2):
                        nc.tensor.matmul(
                            ps1[:, h, :],
                            xt[:, g * I2 + h * 128:g * I2 + (h + 1) * 128],
                            hbd_r,
                            start=True, stop=True,
                        )
                    zt = zpool.tile([128, 2, 128], mm_dt, tag="zt")
                    nc.vector.tensor_copy(zt[:], ps1[:, :, 0:128])
                    ps2 = ps2pool.tile([128, I2], f32, tag="ps2")
                    for h in range(2):
                        nc.tensor.matmul(
                            ps2[:],
                            zt[:, h, :],
                            h256_t[:, h, :],
                            start=(h == 0), stop=(h == 1),
                        )
                    nc.scalar.copy(yt[:, g * I2:(g + 1) * I2], ps2[:])
                # Output DMA on the ACT HWDGE ring so it never blocks the SP
                # ring's input prefetch (HWDGE DMAs are FIFO per issuing engine).
                nc.scalar.dma_start(
                    yr[s],
                    yt[:].rearrange("p (g j) -> p g j", g=G_SUPER),
                )

    nc.compile()
    _CACHED_NC = nc
    return nc


def run(x_np, trace=False):
    """x_np: (..., 4096) fp32, 8192 rows total. Returns (y, exec_time_ns)."""
    x_flat = np.ascontiguousarray(
        np.asarray(x_np, dtype=np.float32).reshape(-1, N_LAST)
    )
    assert x_flat.shape[0] == N_CORES * ROWS_PER_CORE
    hbd_np, h256_np = _build_consts()
    nc = _build_program()
    in_maps = [
        {
            "x": x_flat[c * ROWS_PER_CORE:(c + 1) * ROWS_PER_CORE],
            "hbd": hbd_np,
            "h256": h256_np,
        }
        for c in range(N_CORES)
    ]
    res = run_bass_kernel_spmd(nc, in_maps, list(range(N_CORES)), trace=trace)
    y = np.concatenate([res.results[c]["y"] for c in range(N_CORES)], axis=0)
    return y.reshape(np.asarray(x_np).shape), res.exec_time_ns


def kernel(x):
    x = np.asarray(x)
    y, _ = run(x)
    return y.astype(np.float32)
